# revision 41
# baseline (speedup 1.0000x reference)
"""Trainium2 Bass kernel for nn_Attention_Temp_1468878815458.

Math: the reference computes
    pos   = arange(S) @ Wp.T + bp                       # (S,)
    embed = x.squeeze(1) + pos[:, None]                 # (B,S,D)
    v/k/q = embed @ {Wv,Wk,Wq}.T
    scores[b,x,y]  = (sum_q queries[b,q,x]) * (sum_k keys[b,k,y])
    attention      = softmax(scores, axis=1)            # over x
    out[b,v,y]     = sum_x attention[b,x,y] * sum_n values[b,v,n]

Since softmax normalizes over axis=1 and is then *summed* over axis=1,
sum_x attention[b,x,y] == 1 exactly.  Therefore
    out[b,s,y] = sum_n values[b,s,n]
               = (x[b,0,s,:] + pos[s]) . wv      for every y,
where wv[d] = sum_n Wv[n,d].  The kernel streams x once, computes the
per-row weighted sum with wv, adds the per-s bias pos[s]*sum(wv), and
broadcasts the scalar across the last dim.

Sharding: pure data parallel over batch, 1024 batches per core.  Each
core's shard is viewed as (128 partitions, 6144 values): partition p
holds 64 consecutive rows (8 batches x 8 seq) contiguously -> fully
contiguous DMA in AND out.  x is cast to bf16 on the host so the
in-stream HBM traffic is half of f32 (compute was already bf16).

Device pipeline (per core, chunked over rows-per-partition):
  in-DMA   HWDGE on the SP ring (bf16, no cast needed).  wv AND the
           per-row bias (both bf16) are PREPENDED to x per partition on
           the host, so chunk0's single DMA delivers all constants +
           the first rows in one transfer + one ~0.9us completion
           receipt (no separate const DMAs, no tiny-descriptor traffic
           in the queue; fill measured 3.6 -> ~3.0us)
  DVE      mul by wv (bf16 2x mode ~0.65ns/e), fold 96->48->24 (2x,
           compact contiguous tiles -- strided APs or any f32 operand
           drop TT to 1x), reduce 24->1 (TensorReduce has NO fast mode,
           ~1.3ns/e, so keep its input narrow)
  GPSIMD   per-row bias add only (GPSIMD TT measured ~4.6ns/e -- far too
           slow for bulk work; it also cannot do free-axis reduces)
  ACT      broadcast rowdot across the 96 output columns (~1.05ns/e)
  out-DMA  one eager out-trigger PER CHUNK (measured ~0.8us faster
           than coarse groups), spread across all three DMA paths at
           the tail: chunks 0..3 on the SP HWDGE ring, chunk 4 on the
           GPSIMD SWDGE ring (slower ~1us first-byte, but its data is
           ready earlier), chunk 5 -- the LAST producer -- on the ACT
           HWDGE ring (~0.6us first-byte).  The
           final broadcasts complete nearly together, so three ~0.65us
           triggers on one ring would serialize the tail (measured
           last-compute -> last-byte 3.15 -> 2.73us).  bf16; host
           upcasts to f32
The last DVE_TAIL chunks run bias+broadcast on DVE so ACT's trail does
not gate the kernel tail; their rd is bf16 and their out-blocks are
written TRANSPOSED ([p][d][r], r innermost stride-1 in both operands),
which lifts the final copies from a 1x f32->bf16 CAST (~919ns) into
2x-mode COPYs (~359ns) -- ~0.75us off the end of the DVE chain.  The
host un-transposes those rows when unsharding.

Timing on 8 axon NeuronCores: ~21.3-22.0us in fast phases (was 26.9us
for the f32-input + strided-reduce version).  NOTE: the axon device
alternates between fast (~22us) and slow (~26-28us) phases lasting
minutes; A/B-compare kernels ONLY with compare.py-style interleaved
runs in one process.  Measured negative results (do not retry):
fold-96->48 as a SWDGE CCE accum-DMA (SBUF->SBUF dst+=src, swizzled
mul output) is ~1us WORSE than the DVE fold -- the per-chunk trigger
(650ns) + completion receipt (~900ns) latency lands on the chain;
7-chunk and small-first-chunk variants are neutral-to-worse;
stripping the unused PE engine's whole program (STRIP_PE pass, gather
threshold 4->3) CRASHES the runtime (JaxRuntimeError INTERNAL /
axon_stop_nrt_profile rc=-1) -- the NEFF must carry all 5 engine
programs.
Window anatomy (fast phase, NTFF json, which excludes the first ~3.5us
launch gap): ~3.1us preamble (IRAM loads + prologs), ~1.9us fill,
~10us DVE-bound streaming (HBM floor 8.8us), tail ~1.5us.
"""

import numpy as np

import concourse.bass as bass
import concourse.mybir as mybir
from concourse.bass import broadcast_tensor_aps
from concourse.bass_utils import run_bass_kernel_spmd
from concourse.tile import TileContext

N_CORES = 8
B, S, D = 8192, 8, 96
BPC = B // N_CORES          # 1024 batches per core
ROWS = BPC * S              # 8192 rows of length D per core
P = 128                     # SBUF partitions
FREE = ROWS * D // P        # 6144 bf16 per partition
RPP = ROWS // P             # 64 rows per partition
H = D // 2                  # fold width

# pipeline chunk sizes in rows-per-partition: small first chunk so the
# compute pipeline starts early, big middle chunks to amortize the
# ~130-650ns per-instruction/trigger overheads, tiny last chunk so the
# final out-DMA fires right after the last broadcast
CHUNK_ROWS = [8, 14, 16, 14, 8, 4]
# chunk grouping per out-DMA trigger: big groups early (their data is
# complete mid-stream, bulk out traffic overlaps compute), the last
# chunk alone so the final (tiny) out-DMA fires ASAP
OUT_GROUPS = [(0,), (1,), (2,), (3,), (4,), (5,)]
assert sum(CHUNK_ROWS) == RPP
NCH = len(CHUNK_ROWS)
# experiment: drop the per-engine RegisterMove preamble instructions
STRIP_MOVES = True
# drop the unused PE engine's whole program (IRAM load off the preamble)
STRIP_PE = False
# last DVE_TAIL chunks run bias+broadcast on DVE: ACT's broadcast trail
# (~1.05ns/e + 240ns/op) otherwise gates the kernel tail by ~2us
DVE_TAIL = 2
# chunks whose fold 96->48 runs as a SWDGE CCE accum-DMA (SBUF->SBUF
# dst += src) instead of a DVE tensor_tensor: the mul writes a
# half-swizzled layout so both halves are contiguous 128-desc blocks
FOLD1_DMA = ()

_NC_CACHE = None


def _build() -> bass.Bass:
    # seq codegen lowers multi-wait sync (e.g. the kernel-tail drain) to
    # sequencer commands; this walrus build allows only 1 wait per inst
    nc = bass.Bass(use_seq_codegen=True, enable_partition_id=False)
    # x with wv (bf16, replicated across partitions) prepended per
    # partition: chunk0's single DMA delivers both, so mul0 waits one
    # transfer + one completion receipt instead of two
    # wv AND the per-row bias (both bf16, replicated/tiled across
    # partitions) are prepended to x per partition: chunk0's single DMA
    # delivers all constants + the first rows in one transfer
    x = nc.declare_dram_parameter(
        "x", [P, D + RPP + FREE], mybir.dt.bfloat16, isOutput=False
    )
    # bf16 output halves the out-stream HBM bytes; host upcasts to f32
    out = nc.declare_dram_parameter("out", [P, FREE], mybir.dt.bfloat16, isOutput=True)

    with TileContext(nc) as tc:
        with (
            tc.tile_pool(name="const", bufs=1) as cpool,
            # unique tag per chunk -> each tile gets its own slot: no slot
            # reuse, no WAR waits -> all in-triggers fire back-to-back
            tc.tile_pool(name="xp", bufs=1) as xpool,
            tc.tile_pool(name="pp", bufs=3) as ppool,
            tc.tile_pool(name="fp", bufs=3) as fpool,
            tc.tile_pool(name="gp", bufs=3) as gpool,
            tc.tile_pool(name="op", bufs=1) as opool,
            tc.tile_pool(name="rp", bufs=1) as rpool,
        ):
            # trigger order on the SP HWDGE ring (FIFO queue): wv first
            # (mul0 needs it), then chunk0, then the bias const (only
            # needed after the first reduce -- its 128 tiny descriptors
            # must not sit ahead of chunk0), then the remaining chunks.
            # None of these waits on anything (unique tiles), so the whole
            # in-stream queues immediately and drains at HBM rate.
            CH = D + RPP
            xts = []
            wvh_sb = None
            bias_sb = None
            r0 = 0
            for c, chr_ in enumerate(CHUNK_ROWS):
                chf = chr_ * D
                if c == 0:
                    xt = xpool.tile([P, CH + chf], mybir.dt.bfloat16, tag="xt0")
                    nc.sync.dma_start(out=xt[:], in_=x[:, : CH + chf])
                    wvh_sb = xt[:, :D]
                    bias_sb = xt[:, D:CH]
                    xts.append(xt[:, CH:])
                else:
                    xt = xpool.tile([P, chf], mybir.dt.bfloat16, tag=f"xt{c}")
                    nc.sync.dma_start(
                        out=xt[:], in_=x[:, CH + r0 * D : CH + r0 * D + chf]
                    )
                    xts.append(xt[:])
                r0 += chr_

            r0 = 0
            ot = None
            ot_r0 = 0
            ot_fill = 0
            pending_outs = []
            for c, chr_ in enumerate(CHUNK_ROWS):
                chf = chr_ * D
                tail = c >= NCH - DVE_TAIL
                xta = xts[c]
                pt = ppool.tile([P, chf], mybir.dt.bfloat16, tag="pt")
                hf = chr_ * H
                if c in FOLD1_DMA:
                    # mul writes [p][h][r][48] blocks (iteration order kept
                    # as p,r,h,d so in/out APs walk in lockstep); fold1 is
                    # then one accum-DMA: block0 += block1, 128 contiguous
                    # descriptors, off the DVE entirely
                    x3 = xta.rearrange("p (r h d) -> p r h d", h=2, d=H)
                    wv4 = wvh_sb.rearrange("p (r h d) -> p r h d", r=1, h=2)
                    _, wv4b = broadcast_tensor_aps(x3, wv4)
                    p4 = pt[:, :chf].rearrange("p (h r d) -> p r h d", h=2, d=H)
                    nc.vector.tensor_tensor(
                        out=p4, in0=x3, in1=wv4b, op=mybir.AluOpType.mult
                    )
                    nc.gpsimd.dma_start(
                        out=pt[:, :hf],
                        in_=pt[:, hf : 2 * hf],
                        accum_op=mybir.AluOpType.add,
                    )
                    f3 = pt[:, :hf].rearrange("p (r d) -> p r d", d=H)
                else:
                    x3 = xta.rearrange("p (r d) -> p r d", d=D)
                    wv3 = wvh_sb.rearrange("p (r d) -> p r d", r=1)
                    _, wv3b = broadcast_tensor_aps(x3, wv3)
                    p3 = pt[:, :chf].rearrange("p (r d) -> p r d", d=D)
                    nc.vector.tensor_tensor(
                        out=p3, in0=x3, in1=wv3b, op=mybir.AluOpType.mult
                    )
                    # fold 96 -> 48 into a compact tile (contiguous output
                    # keeps the op in 2x mode and the fold-2 input packed)
                    ft = fpool.tile([P, hf], mybir.dt.bfloat16, tag="ft")
                    f3 = ft[:, :hf].rearrange("p (r d) -> p r d", d=H)
                    nc.vector.tensor_tensor(
                        out=f3, in0=p3[:, :, :H], in1=p3[:, :, H:],
                        op=mybir.AluOpType.add,
                    )
                # fold 48 -> 24 (GPSIMD measured ~4.6ns/e for TT -- far too
                # slow and it stalls the chain; keep all folds on DVE)
                Q = H // 2
                gt = gpool.tile([P, chr_ * Q], mybir.dt.bfloat16, tag="gt")
                g3 = gt[:, : chr_ * Q].rearrange("p (r d) -> p r d", d=Q)
                nc.vector.tensor_tensor(
                    out=g3, in0=f3[:, :, :Q], in1=f3[:, :, Q:], op=mybir.AluOpType.add
                )

                # reduce 24 -> 1 per row (DVE only; no fast mode) + bias.
                # Tail rd is bf16: the tail broadcast then has all-2-byte
                # operands, qualifying for the DVE 2x/4x copy modes
                rdt = mybir.dt.bfloat16 if tail else mybir.dt.float32
                rd = rpool.tile([P, chr_], rdt, tag=f"rd{c}")
                if tail:
                    # HW reduce accumulates internally in fp32; only the
                    # stored result rounds to bf16 (output is bf16 anyway)
                    with nc.allow_low_precision(reason="bf16 out stream"):
                        nc.vector.reduce_sum(
                            out=rd[:], in_=g3, axis=mybir.AxisListType.X
                        )
                else:
                    nc.vector.reduce_sum(
                        out=rd[:], in_=g3, axis=mybir.AxisListType.X
                    )
                bias_eng = nc.vector if tail else nc.gpsimd
                bias_eng.tensor_add(
                    out=rd[:], in0=rd[:], in1=bias_sb[:, r0 : r0 + chr_]
                )

                grp = next(g for g in OUT_GROUPS if c in g)
                if ot is None:
                    grp_free = sum(CHUNK_ROWS[j] for j in grp) * D
                    ot = opool.tile([P, grp_free], mybir.dt.bfloat16, tag=f"ot{c}")
                    ot_r0 = r0
                    ot_fill = 0
                if tail:
                    # transposed block layout [p][d][r]: innermost dim of
                    # src AND dst is the r-run (stride 1, 2-byte) -> the
                    # copy runs in a DVE fast mode instead of a 1x CAST.
                    # The host un-transposes these rows when unsharding
                    otT = ot[:, ot_fill : ot_fill + chf].rearrange(
                        "p (d r) -> p d r", r=chr_
                    )
                    rdT = rd[:].rearrange("p (d r) -> p d r", d=1)
                    _, rdTb = broadcast_tensor_aps(otT, rdT)
                    nc.vector.tensor_copy(out=otT, in_=rdTb)
                else:
                    ot3 = ot[:, ot_fill : ot_fill + chf].rearrange(
                        "p (r d) -> p r d", d=D
                    )
                    rd3 = rd[:].rearrange("p (r d) -> p r d", d=1)
                    _, rd3b = broadcast_tensor_aps(ot3, rd3)
                    nc.scalar.copy(out=ot3, in_=rd3b)
                ot_fill += chf
                r0 += chr_

                if c == grp[-1]:
                    # deferred to the end of the build: the SP HWDGE ring is
                    # FIFO per issuing engine, so a waiting out-trigger must
                    # sit behind ALL (wait-free) in-triggers.  The last two
                    # groups go out on the ACT HWDGE ring and the GPSIMD
                    # SWDGE ring: the final broadcasts complete nearly
                    # together, and three ~0.65us triggers on one ring
                    # would serialize the tail
                    pending_outs.append(
                        (c, out[:, ot_r0 * D : ot_r0 * D + ot_fill], ot[:, :ot_fill])
                    )
                    ot = None
            for c, dst, src in pending_outs:
                # the LAST-produced chunk rides the ACT HWDGE ring
                # (first-byte ~0.6us) and the second-to-last takes the
                # slower SWDGE path (~1us first-byte) -- it has slack
                if c == NCH - 1:
                    eng = nc.scalar
                elif c == NCH - 2:
                    eng = nc.gpsimd
                else:
                    eng = nc.sync
                eng.dma_start(out=dst, in_=src)
    _strip_unused_const_memsets(nc)
    _split_multi_waits(nc)
    _trim_tail_barrier(nc)
    if STRIP_MOVES:
        _strip_register_moves(nc)
    if STRIP_PE:
        _strip_pe(nc)
    return nc


def _strip_pe(nc: bass.Bass) -> None:
    """Remove the (unused) PE engine's program entirely.

    PE executes nothing in the body; it only contributes +1 to the
    kernel-tail barrier's gather sem.  Dropping its program removes its
    ~1.2KB IRAM load from the trickling instruction-fetch queue in the
    preamble.  The Pool-side gather threshold is lowered 4 -> 3 to
    match (PE's release re-increment feeds nothing: the second tail
    barrier is already trimmed)."""
    for f in nc.m.functions:
        for bb in f.blocks:
            bb.instructions[:] = [
                i for i in bb.instructions
                if getattr(i, "engine", None) != mybir.EngineType.PE
            ]
            for i in bb.instructions:
                si = i.sync_info
                if not si:
                    continue
                for s in si.on_wait or []:
                    if (
                        s.ant_name == "barrier_Pool_Activation_PE_DVE_SP_gather"
                        and s.wait_value == 4
                    ):
                        s.wait_value = 3


def _strip_register_moves(nc: bass.Bass) -> None:
    """Drop the per-engine InstRegisterMove preamble (~0.5us serial per
    engine before the first body instruction)."""
    for f in nc.m.functions:
        for bb in f.blocks:
            if bb.name != "main":
                continue
            bb.instructions[:] = [
                i for i in bb.instructions
                if not isinstance(i, mybir.InstRegisterMove)
            ]


def _trim_tail_barrier(nc: bass.Bass) -> None:
    """The kernel tail is: drain -> all-engine barrier -> sem-clear ->
    all-engine barrier.  The second barrier only orders the sem-clear
    against a *next* invocation, which NRT already serializes on NEFF
    completion (every sequencer, including Pool after the clear, must
    retire).  Dropping it removes ~1us from the measured exec window."""
    for f in nc.m.functions:
        bb = f.blocks[-1]
        last_isa = None
        for i, inst in enumerate(bb.instructions):
            if isinstance(inst, mybir.InstISA):
                last_isa = i
        if last_isa is not None:
            del bb.instructions[last_isa + 1 :]


def _strip_unused_const_memsets(nc: bass.Bass) -> None:
    """Bass unconditionally memsets 4 const SBUF tensors on GPSIMD in the
    preamble (~3us on the init-barrier critical path).  This kernel never
    reads them; drop the memsets.  The init all-engine barrier that
    followed them is also dead once they're gone: engines are independent
    until the Tile-emitted semaphores in the body, and NRT guarantees a
    clean sem state at NEFF start."""
    for f in nc.m.functions:
        for bb in f.blocks:
            if bb.name != "main":
                continue
            keep = []
            for inst in bb.instructions:
                if isinstance(
                    inst, mybir.InstMemset | mybir.InstDrain | mybir.InstEventSemaphore
                ):
                    continue
                keep.append(inst)
            if len(keep) != len(bb.instructions):
                bb.instructions[:] = keep


def _split_multi_waits(nc: bass.Bass) -> None:
    """Walrus (this build) allows only one sync wait per instruction.

    Tile's kernel-tail drain merges waits on every DMA lane + engine sem
    into one instruction; split the extras onto same-engine NOPs placed
    immediately before it.
    """
    for f in nc.m.functions:
        for bb in f.blocks:
            insts = bb.instructions
            i = 0
            while i < len(insts):
                inst = insts[i]
                si = inst.sync_info
                if si is not None and si.on_wait and len(si.on_wait) > 1:
                    waits = list(si.on_wait)
                    nops = []
                    for j, w in enumerate(waits[:-1]):
                        nop = mybir.InstNoOp(
                            name=f"{inst.name}-wsplit{j}", ins=[], outs=[]
                        )
                        nop.engine = inst.engine
                        nop.sync_info = mybir.SyncInfo(on_wait=[w], on_update=[])
                        nc.register_instruction(nop)
                        nops.append(nop)
                    inst.sync_info = mybir.SyncInfo(
                        on_wait=[waits[-1]], on_update=list(si.on_update)
                    )
                    insts[i:i] = nops
                    i += len(nops)
                i += 1
    return


def _get_nc() -> bass.Bass:
    global _NC_CACHE
    if _NC_CACHE is None:
        _NC_CACHE = _build()
    return _NC_CACHE


def _make_in_maps(x, Wp, bp, Wv):
    import ml_dtypes

    x = np.asarray(x, dtype=np.float32)
    Wp = np.asarray(Wp, dtype=np.float32)
    bp = np.asarray(bp, dtype=np.float32)
    Wv = np.asarray(Wv, dtype=np.float32)

    # fold the tiny weights (O(D^2) host prep)
    p = np.arange(S, dtype=np.float32)
    pos = p @ Wp.T + bp                       # (S,)
    wv = Wv.sum(axis=0)                       # (D,) column sums
    bias8 = (pos * wv.sum()).astype(np.float32)
    bias_rpp = np.tile(bias8, RPP // S)       # (RPP,) pattern per in-partition row
    cb_row = np.concatenate([wv, bias_rpp]).astype(ml_dtypes.bfloat16)
    cb = np.broadcast_to(cb_row, (P, D + RPP))

    xh = x.reshape(B * S * D).astype(ml_dtypes.bfloat16)
    in_maps = []
    for i in range(N_CORES):
        shard = xh[i * ROWS * D : (i + 1) * ROWS * D].reshape(P, FREE)
        xplus = np.ascontiguousarray(np.concatenate([cb, shard], axis=1))
        in_maps.append({"x": xplus})
    return in_maps


def _run(x, Wp, bp, Wv, trace=False, **spmd_kwargs):
    nc = _get_nc()
    in_maps = _make_in_maps(x, Wp, bp, Wv)
    res = run_bass_kernel_spmd(
        nc, in_maps, list(range(N_CORES)), trace=trace, **spmd_kwargs
    )
    tail_chunks = CHUNK_ROWS[NCH - DVE_TAIL:]
    head_rows = RPP - sum(tail_chunks)
    parts = []
    for i in range(N_CORES):
        full = np.asarray(res.results[i]["out"]).astype(np.float32)
        blocks = [full[:, : head_rows * D].reshape(P, head_rows, D)]
        off = head_rows * D
        for chr_ in tail_chunks:
            blocks.append(
                full[:, off : off + chr_ * D]
                .reshape(P, D, chr_)
                .transpose(0, 2, 1)
            )
            off += chr_ * D
        parts.append(
            np.concatenate(blocks, axis=1).reshape(BPC, S, D)
        )
    return np.concatenate(parts, axis=0), res


def kernel(x, Wp, bp, Wv, Wk, Wq) -> np.ndarray:
    out, _ = _run(x, Wp, bp, Wv)
    return out


# revision 42
# speedup vs baseline: 1.1342x; 1.1342x over previous
"""Trainium2 Bass kernel for nn_Attention_Temp_1468878815458.

Math: the reference computes
    pos   = arange(S) @ Wp.T + bp                       # (S,)
    embed = x.squeeze(1) + pos[:, None]                 # (B,S,D)
    v/k/q = embed @ {Wv,Wk,Wq}.T
    scores[b,x,y]  = (sum_q queries[b,q,x]) * (sum_k keys[b,k,y])
    attention      = softmax(scores, axis=1)            # over x
    out[b,v,y]     = sum_x attention[b,x,y] * sum_n values[b,v,n]

Since softmax normalizes over axis=1 and is then *summed* over axis=1,
sum_x attention[b,x,y] == 1 exactly.  Therefore
    out[b,s,y] = sum_n values[b,s,n]
               = (x[b,0,s,:] + pos[s]) . wv      for every y,
where wv[d] = sum_n Wv[n,d].  The kernel streams x once, computes the
per-row weighted sum with wv, adds the per-s bias pos[s]*sum(wv), and
broadcasts the scalar across the last dim.

Sharding: pure data parallel over batch, 1024 batches per core.  Each
core's shard is viewed as (128 partitions, 6144 values): partition p
holds 64 consecutive rows (8 batches x 8 seq) contiguously -> fully
contiguous DMA in AND out.  x is cast to bf16 on the host so the
in-stream HBM traffic is half of f32 (compute was already bf16).

Device pipeline (per core, chunked over rows-per-partition):
  in-DMA   HWDGE on the SP ring (bf16, no cast needed).  wv AND the
           per-row bias (both bf16) are PREPENDED to x per partition on
           the host, so chunk0's single DMA delivers all constants +
           the first rows in one transfer + one ~0.9us completion
           receipt (no separate const DMAs, no tiny-descriptor traffic
           in the queue; fill measured 3.6 -> ~3.0us)
  DVE      mul by wv (bf16 2x mode ~0.65ns/e), fold 96->48->24 (2x,
           compact contiguous tiles -- strided APs or any f32 operand
           drop TT to 1x), reduce 24->1 (TensorReduce has NO fast mode,
           ~1.3ns/e, so keep its input narrow)
  GPSIMD   per-row bias add only (GPSIMD TT measured ~4.6ns/e -- far too
           slow for bulk work; it also cannot do free-axis reduces)
  ACT      broadcast rowdot across the 96 output columns (~1.05ns/e)
  out-DMA  one eager out-trigger PER CHUNK (measured ~0.8us faster
           than coarse groups), spread across all three DMA paths at
           the tail: chunks 0..3 on the SP HWDGE ring, chunk 4 on the
           GPSIMD SWDGE ring (slower ~1us first-byte, but its data is
           ready earlier), chunk 5 -- the LAST producer -- on the ACT
           HWDGE ring (~0.6us first-byte).  The
           final broadcasts complete nearly together, so three ~0.65us
           triggers on one ring would serialize the tail (measured
           last-compute -> last-byte 3.15 -> 2.73us).  bf16; host
           upcasts to f32
The last DVE_TAIL chunks run bias+broadcast on DVE so ACT's trail does
not gate the kernel tail; their rd is bf16 and their out-blocks are
written TRANSPOSED ([p][d][r], r innermost stride-1 in both operands),
which lifts the final copies from a 1x f32->bf16 CAST (~919ns) into
2x-mode COPYs (~359ns) -- ~0.75us off the end of the DVE chain.  The
host un-transposes those rows when unsharding.

Timing on 8 axon NeuronCores: ~21.3-22.0us in fast phases (was 26.9us
for the f32-input + strided-reduce version).  NOTE: the axon device
alternates between fast (~22us) and slow (~26-28us) phases lasting
minutes; A/B-compare kernels ONLY with compare.py-style interleaved
runs in one process.  Measured negative results (do not retry):
fold-96->48 as a SWDGE CCE accum-DMA (SBUF->SBUF dst+=src, swizzled
mul output) is ~1us WORSE than the DVE fold -- the per-chunk trigger
(650ns) + completion receipt (~900ns) latency lands on the chain;
7-chunk and small-first-chunk variants are neutral-to-worse;
stripping the unused PE engine's whole program (STRIP_PE pass, gather
threshold 4->3) CRASHES the runtime (JaxRuntimeError INTERNAL /
axon_stop_nrt_profile rc=-1) -- the NEFF must carry all 5 engine
programs.
Window anatomy (fast phase, NTFF json, which excludes the first ~3.5us
launch gap): ~3.1us preamble (IRAM loads + prologs), ~1.9us fill,
~10us DVE-bound streaming (HBM floor 8.8us), tail ~1.5us.
"""

import numpy as np

import concourse.bass as bass
import concourse.mybir as mybir
from concourse.bass import broadcast_tensor_aps
from concourse.bass_utils import run_bass_kernel_spmd
from concourse.tile import TileContext

N_CORES = 8
B, S, D = 8192, 8, 96
BPC = B // N_CORES          # 1024 batches per core
ROWS = BPC * S              # 8192 rows of length D per core
P = 128                     # SBUF partitions
FREE = ROWS * D // P        # 6144 bf16 per partition
RPP = ROWS // P             # 64 rows per partition
H = D // 2                  # fold width

# pipeline chunk sizes in rows-per-partition: small first chunk so the
# compute pipeline starts early, big middle chunks to amortize the
# ~130-650ns per-instruction/trigger overheads, tiny last chunk so the
# final out-DMA fires right after the last broadcast
CHUNK_ROWS = [8, 14, 16, 14, 8, 4]
# chunk grouping per out-DMA trigger: big groups early (their data is
# complete mid-stream, bulk out traffic overlaps compute), the last
# chunk alone so the final (tiny) out-DMA fires ASAP
OUT_GROUPS = [(0,), (1,), (2,), (3,), (4,), (5,)]
assert sum(CHUNK_ROWS) == RPP
NCH = len(CHUNK_ROWS)
# experiment: drop the per-engine RegisterMove preamble instructions
STRIP_MOVES = True
# drop the unused PE engine's whole program (IRAM load off the preamble)
STRIP_PE = False
# last DVE_TAIL chunks run bias+broadcast on DVE: ACT's broadcast trail
# (~1.05ns/e + 240ns/op) otherwise gates the kernel tail by ~2us
DVE_TAIL = 2
# chunks whose fold 96->48 runs as a SWDGE CCE accum-DMA (SBUF->SBUF
# dst += src) instead of a DVE tensor_tensor: the mul writes a
# half-swizzled layout so both halves are contiguous 128-desc blocks
FOLD1_DMA = ()

_NC_CACHE = None


def _build() -> bass.Bass:
    # seq codegen lowers multi-wait sync (e.g. the kernel-tail drain) to
    # sequencer commands; this walrus build allows only 1 wait per inst
    nc = bass.Bass(use_seq_codegen=True, enable_partition_id=False)
    # x with wv (bf16, replicated across partitions) prepended per
    # partition: chunk0's single DMA delivers both, so mul0 waits one
    # transfer + one completion receipt instead of two
    # wv AND the per-row bias (both bf16, replicated/tiled across
    # partitions) are prepended to x per partition: chunk0's single DMA
    # delivers all constants + the first rows in one transfer
    x = nc.declare_dram_parameter(
        "x", [P, D + RPP + FREE], mybir.dt.bfloat16, isOutput=False
    )
    # bf16 output halves the out-stream HBM bytes; host upcasts to f32
    out = nc.declare_dram_parameter("out", [P, FREE], mybir.dt.bfloat16, isOutput=True)

    with TileContext(nc) as tc:
        with (
            tc.tile_pool(name="const", bufs=1) as cpool,
            # unique tag per chunk -> each tile gets its own slot: no slot
            # reuse, no WAR waits -> all in-triggers fire back-to-back
            tc.tile_pool(name="xp", bufs=1) as xpool,
            tc.tile_pool(name="pp", bufs=3) as ppool,
            tc.tile_pool(name="fp", bufs=3) as fpool,
            tc.tile_pool(name="gp", bufs=3) as gpool,
            tc.tile_pool(name="op", bufs=1) as opool,
            tc.tile_pool(name="rp", bufs=1) as rpool,
        ):
            # trigger order on the SP HWDGE ring (FIFO queue): wv first
            # (mul0 needs it), then chunk0, then the bias const (only
            # needed after the first reduce -- its 128 tiny descriptors
            # must not sit ahead of chunk0), then the remaining chunks.
            # None of these waits on anything (unique tiles), so the whole
            # in-stream queues immediately and drains at HBM rate.
            CH = D + RPP
            xts = []
            wvh_sb = None
            bias_sb = None
            r0 = 0
            for c, chr_ in enumerate(CHUNK_ROWS):
                chf = chr_ * D
                if c == 0:
                    xt = xpool.tile([P, CH + chf], mybir.dt.bfloat16, tag="xt0")
                    nc.sync.dma_start(out=xt[:], in_=x[:, : CH + chf])
                    wvh_sb = xt[:, :D]
                    bias_sb = xt[:, D:CH]
                    xts.append(xt[:, CH:])
                else:
                    xt = xpool.tile([P, chf], mybir.dt.bfloat16, tag=f"xt{c}")
                    nc.sync.dma_start(
                        out=xt[:], in_=x[:, CH + r0 * D : CH + r0 * D + chf]
                    )
                    xts.append(xt[:])
                r0 += chr_

            r0 = 0
            ot = None
            ot_r0 = 0
            ot_fill = 0
            pending_outs = []
            for c, chr_ in enumerate(CHUNK_ROWS):
                chf = chr_ * D
                tail = c >= NCH - DVE_TAIL
                xta = xts[c]
                pt = ppool.tile([P, chf], mybir.dt.bfloat16, tag="pt")
                hf = chr_ * H
                if c in FOLD1_DMA:
                    # mul writes [p][h][r][48] blocks (iteration order kept
                    # as p,r,h,d so in/out APs walk in lockstep); fold1 is
                    # then one accum-DMA: block0 += block1, 128 contiguous
                    # descriptors, off the DVE entirely
                    x3 = xta.rearrange("p (r h d) -> p r h d", h=2, d=H)
                    wv4 = wvh_sb.rearrange("p (r h d) -> p r h d", r=1, h=2)
                    _, wv4b = broadcast_tensor_aps(x3, wv4)
                    p4 = pt[:, :chf].rearrange("p (h r d) -> p r h d", h=2, d=H)
                    nc.vector.tensor_tensor(
                        out=p4, in0=x3, in1=wv4b, op=mybir.AluOpType.mult
                    )
                    nc.gpsimd.dma_start(
                        out=pt[:, :hf],
                        in_=pt[:, hf : 2 * hf],
                        accum_op=mybir.AluOpType.add,
                    )
                    f3 = pt[:, :hf].rearrange("p (r d) -> p r d", d=H)
                else:
                    x3 = xta.rearrange("p (r d) -> p r d", d=D)
                    wv3 = wvh_sb.rearrange("p (r d) -> p r d", r=1)
                    _, wv3b = broadcast_tensor_aps(x3, wv3)
                    p3 = pt[:, :chf].rearrange("p (r d) -> p r d", d=D)
                    nc.vector.tensor_tensor(
                        out=p3, in0=x3, in1=wv3b, op=mybir.AluOpType.mult
                    )
                    # fold 96 -> 48 into a compact tile (contiguous output
                    # keeps the op in 2x mode and the fold-2 input packed)
                    ft = fpool.tile([P, hf], mybir.dt.bfloat16, tag="ft")
                    f3 = ft[:, :hf].rearrange("p (r d) -> p r d", d=H)
                    nc.vector.tensor_tensor(
                        out=f3, in0=p3[:, :, :H], in1=p3[:, :, H:],
                        op=mybir.AluOpType.add,
                    )
                # fold 48 -> 24 (GPSIMD measured ~4.6ns/e for TT -- far too
                # slow and it stalls the chain; keep all folds on DVE).
                # The last chunk skips BOTH folds: at 4 rows the ~150ns
                # per-op setups outweigh the reduce width penalty, and
                # two fewer ops sit on the very end of the kernel chain
                last = c == NCH - 1
                if not last:
                    Q = H // 2
                    gt = gpool.tile([P, chr_ * Q], mybir.dt.bfloat16, tag="gt")
                    g3 = gt[:, : chr_ * Q].rearrange("p (r d) -> p r d", d=Q)
                    nc.vector.tensor_tensor(
                        out=g3, in0=f3[:, :, :Q], in1=f3[:, :, Q:],
                        op=mybir.AluOpType.add,
                    )

                # reduce 24 -> 1 per row (DVE only; no fast mode) + bias.
                # Tail rd is bf16: the tail broadcast then has all-2-byte
                # operands, qualifying for the DVE 2x/4x copy modes
                red_in = p3 if last else g3
                rdt = mybir.dt.bfloat16 if tail else mybir.dt.float32
                rd = rpool.tile([P, chr_], rdt, tag=f"rd{c}")
                if tail:
                    # HW reduce accumulates internally in fp32; only the
                    # stored result rounds to bf16 (output is bf16 anyway)
                    with nc.allow_low_precision(reason="bf16 out stream"):
                        nc.vector.reduce_sum(
                            out=rd[:], in_=red_in, axis=mybir.AxisListType.X
                        )
                else:
                    nc.vector.reduce_sum(
                        out=rd[:], in_=red_in, axis=mybir.AxisListType.X
                    )
                bias_eng = nc.vector if tail else nc.gpsimd
                bias_eng.tensor_add(
                    out=rd[:], in0=rd[:], in1=bias_sb[:, r0 : r0 + chr_]
                )

                grp = next(g for g in OUT_GROUPS if c in g)
                if ot is None:
                    grp_free = sum(CHUNK_ROWS[j] for j in grp) * D
                    ot = opool.tile([P, grp_free], mybir.dt.bfloat16, tag=f"ot{c}")
                    ot_r0 = r0
                    ot_fill = 0
                if tail:
                    # transposed block layout [p][d][r]: innermost dim of
                    # src AND dst is the r-run (stride 1, 2-byte) -> the
                    # copy runs in a DVE fast mode instead of a 1x CAST.
                    # The host un-transposes these rows when unsharding
                    otT = ot[:, ot_fill : ot_fill + chf].rearrange(
                        "p (d r) -> p d r", r=chr_
                    )
                    rdT = rd[:].rearrange("p (d r) -> p d r", d=1)
                    _, rdTb = broadcast_tensor_aps(otT, rdT)
                    nc.vector.tensor_copy(out=otT, in_=rdTb)
                else:
                    ot3 = ot[:, ot_fill : ot_fill + chf].rearrange(
                        "p (r d) -> p r d", d=D
                    )
                    rd3 = rd[:].rearrange("p (r d) -> p r d", d=1)
                    _, rd3b = broadcast_tensor_aps(ot3, rd3)
                    nc.scalar.copy(out=ot3, in_=rd3b)
                ot_fill += chf
                r0 += chr_

                if c == grp[-1]:
                    # deferred to the end of the build: the SP HWDGE ring is
                    # FIFO per issuing engine, so a waiting out-trigger must
                    # sit behind ALL (wait-free) in-triggers.  The last two
                    # groups go out on the ACT HWDGE ring and the GPSIMD
                    # SWDGE ring: the final broadcasts complete nearly
                    # together, and three ~0.65us triggers on one ring
                    # would serialize the tail
                    pending_outs.append(
                        (c, out[:, ot_r0 * D : ot_r0 * D + ot_fill], ot[:, :ot_fill])
                    )
                    ot = None
            for c, dst, src in pending_outs:
                # the LAST-produced chunk rides the ACT HWDGE ring
                # (first-byte ~0.6us) and the second-to-last takes the
                # slower SWDGE path (~1us first-byte) -- it has slack
                if c == NCH - 1:
                    eng = nc.scalar
                elif c == NCH - 2:
                    eng = nc.gpsimd
                else:
                    eng = nc.sync
                eng.dma_start(out=dst, in_=src)
    _strip_unused_const_memsets(nc)
    _split_multi_waits(nc)
    _trim_tail_barrier(nc)
    if STRIP_MOVES:
        _strip_register_moves(nc)
    if STRIP_PE:
        _strip_pe(nc)
    return nc


def _strip_pe(nc: bass.Bass) -> None:
    """Remove the (unused) PE engine's program entirely.

    PE executes nothing in the body; it only contributes +1 to the
    kernel-tail barrier's gather sem.  Dropping its program removes its
    ~1.2KB IRAM load from the trickling instruction-fetch queue in the
    preamble.  The Pool-side gather threshold is lowered 4 -> 3 to
    match (PE's release re-increment feeds nothing: the second tail
    barrier is already trimmed)."""
    for f in nc.m.functions:
        for bb in f.blocks:
            bb.instructions[:] = [
                i for i in bb.instructions
                if getattr(i, "engine", None) != mybir.EngineType.PE
            ]
            for i in bb.instructions:
                si = i.sync_info
                if not si:
                    continue
                for s in si.on_wait or []:
                    if (
                        s.ant_name == "barrier_Pool_Activation_PE_DVE_SP_gather"
                        and s.wait_value == 4
                    ):
                        s.wait_value = 3


def _strip_register_moves(nc: bass.Bass) -> None:
    """Drop the per-engine InstRegisterMove preamble (~0.5us serial per
    engine before the first body instruction)."""
    for f in nc.m.functions:
        for bb in f.blocks:
            if bb.name != "main":
                continue
            bb.instructions[:] = [
                i for i in bb.instructions
                if not isinstance(i, mybir.InstRegisterMove)
            ]


def _trim_tail_barrier(nc: bass.Bass) -> None:
    """The kernel tail is: drain -> all-engine barrier -> sem-clear ->
    all-engine barrier.  The second barrier only orders the sem-clear
    against a *next* invocation, which NRT already serializes on NEFF
    completion (every sequencer, including Pool after the clear, must
    retire).  Dropping it removes ~1us from the measured exec window."""
    for f in nc.m.functions:
        bb = f.blocks[-1]
        last_isa = None
        for i, inst in enumerate(bb.instructions):
            if isinstance(inst, mybir.InstISA):
                last_isa = i
        if last_isa is not None:
            del bb.instructions[last_isa + 1 :]


def _strip_unused_const_memsets(nc: bass.Bass) -> None:
    """Bass unconditionally memsets 4 const SBUF tensors on GPSIMD in the
    preamble (~3us on the init-barrier critical path).  This kernel never
    reads them; drop the memsets.  The init all-engine barrier that
    followed them is also dead once they're gone: engines are independent
    until the Tile-emitted semaphores in the body, and NRT guarantees a
    clean sem state at NEFF start."""
    for f in nc.m.functions:
        for bb in f.blocks:
            if bb.name != "main":
                continue
            keep = []
            for inst in bb.instructions:
                if isinstance(
                    inst, mybir.InstMemset | mybir.InstDrain | mybir.InstEventSemaphore
                ):
                    continue
                keep.append(inst)
            if len(keep) != len(bb.instructions):
                bb.instructions[:] = keep


def _split_multi_waits(nc: bass.Bass) -> None:
    """Walrus (this build) allows only one sync wait per instruction.

    Tile's kernel-tail drain merges waits on every DMA lane + engine sem
    into one instruction; split the extras onto same-engine NOPs placed
    immediately before it.
    """
    for f in nc.m.functions:
        for bb in f.blocks:
            insts = bb.instructions
            i = 0
            while i < len(insts):
                inst = insts[i]
                si = inst.sync_info
                if si is not None and si.on_wait and len(si.on_wait) > 1:
                    waits = list(si.on_wait)
                    nops = []
                    for j, w in enumerate(waits[:-1]):
                        nop = mybir.InstNoOp(
                            name=f"{inst.name}-wsplit{j}", ins=[], outs=[]
                        )
                        nop.engine = inst.engine
                        nop.sync_info = mybir.SyncInfo(on_wait=[w], on_update=[])
                        nc.register_instruction(nop)
                        nops.append(nop)
                    inst.sync_info = mybir.SyncInfo(
                        on_wait=[waits[-1]], on_update=list(si.on_update)
                    )
                    insts[i:i] = nops
                    i += len(nops)
                i += 1
    return


def _get_nc() -> bass.Bass:
    global _NC_CACHE
    if _NC_CACHE is None:
        _NC_CACHE = _build()
    return _NC_CACHE


def _make_in_maps(x, Wp, bp, Wv):
    import ml_dtypes

    x = np.asarray(x, dtype=np.float32)
    Wp = np.asarray(Wp, dtype=np.float32)
    bp = np.asarray(bp, dtype=np.float32)
    Wv = np.asarray(Wv, dtype=np.float32)

    # fold the tiny weights (O(D^2) host prep)
    p = np.arange(S, dtype=np.float32)
    pos = p @ Wp.T + bp                       # (S,)
    wv = Wv.sum(axis=0)                       # (D,) column sums
    bias8 = (pos * wv.sum()).astype(np.float32)
    bias_rpp = np.tile(bias8, RPP // S)       # (RPP,) pattern per in-partition row
    cb_row = np.concatenate([wv, bias_rpp]).astype(ml_dtypes.bfloat16)
    cb = np.broadcast_to(cb_row, (P, D + RPP))

    xh = x.reshape(B * S * D).astype(ml_dtypes.bfloat16)
    in_maps = []
    for i in range(N_CORES):
        shard = xh[i * ROWS * D : (i + 1) * ROWS * D].reshape(P, FREE)
        xplus = np.ascontiguousarray(np.concatenate([cb, shard], axis=1))
        in_maps.append({"x": xplus})
    return in_maps


def _run(x, Wp, bp, Wv, trace=False, **spmd_kwargs):
    nc = _get_nc()
    in_maps = _make_in_maps(x, Wp, bp, Wv)
    res = run_bass_kernel_spmd(
        nc, in_maps, list(range(N_CORES)), trace=trace, **spmd_kwargs
    )
    tail_chunks = CHUNK_ROWS[NCH - DVE_TAIL:]
    head_rows = RPP - sum(tail_chunks)
    parts = []
    for i in range(N_CORES):
        full = np.asarray(res.results[i]["out"]).astype(np.float32)
        blocks = [full[:, : head_rows * D].reshape(P, head_rows, D)]
        off = head_rows * D
        for chr_ in tail_chunks:
            blocks.append(
                full[:, off : off + chr_ * D]
                .reshape(P, D, chr_)
                .transpose(0, 2, 1)
            )
            off += chr_ * D
        parts.append(
            np.concatenate(blocks, axis=1).reshape(BPC, S, D)
        )
    return np.concatenate(parts, axis=0), res


def kernel(x, Wp, bp, Wv, Wk, Wq) -> np.ndarray:
    out, _ = _run(x, Wp, bp, Wv)
    return out
